# revision 1
# baseline (speedup 1.0000x reference)
"""Bahdanau attention on 8 TRN2 NeuronCores — data-parallel over batch.

Self-contained: builds one SPMD Bass/Tile program, shards the batch (B=64)
across 8 cores (8 batches/core), replicates the small params, runs via
run_bass_kernel_spmd, and reassembles full outputs (context [64,512],
attn [64,2048]).

Per-core pipeline per batch (big matmuls in float32r — full-speed fp32 mode
with ~TF32 precision; end-to-end rel err ~2e-4 vs fp32 reference):
  1. DMA enc -> fp32 tiles [128(s), D], DVE-cast to fp32r (encN)
  2. PE-transpose -> encT fp32r tiles [128(d), 512(s)]
  3. featT[u,s] = tanh(W1.T @ encT + projdec[b]): PE matmul + ACT tanh(bias)
  4. score row = V.T @ featT (PE, M=1); exp via ACT -> escore [1,512] x 4
  5. PE row-transposes escore -> columns [128(s), 16]; denominator via
     ones-matmul + DVE reciprocal; normalize rows -> attn out
  6. context = escore-cols.T @ encN (PE accumulate), scaled by 1/total
"""
import contextlib

import numpy as np

import concourse.bass as bass
import concourse.tile as tile
from concourse import mybir
from concourse.bass_utils import run_bass_kernel_spmd

F32 = mybir.dt.float32
F32R = mybir.dt.float32r
AF = mybir.ActivationFunctionType
AX = mybir.AxisListType

N_CORES = 8
B, S, D_ENC, D_DEC, UNITS = 64, 2048, 512, 512, 512
B_LOC = B // N_CORES

_NOP_LIMIT = 1


def _wait_limit(ins) -> int:
    return 1


def _legalize_sync_waits(nc) -> int:
    """Split excess per-instruction sync waits onto same-engine NOPs.

    This walrus build accepts very few sync-wait commands per instruction
    (fp32 matmul weight-load: 1; control NOP: 1; DMA trigger: 1). Tile can
    emit more. Extra waits are hoisted onto single-wait NOPs placed
    immediately before the instruction on the same engine — the sequencer
    blocks on each NOP first, preserving semantics.
    """
    n_split = 0
    uid = [0]
    for f in nc.m.functions:
        for bb in f.blocks:
            insts = list(bb.instructions)
            out = []
            changed = False
            for ins in insts:
                si = ins.sync_info
                waits = list(si.on_wait) if si is not None and si.on_wait else []
                limit = _wait_limit(ins)
                if len(waits) > limit:
                    keep = waits[len(waits) - limit:]
                    extra = waits[: len(waits) - limit]
                    for k in range(0, len(extra), _NOP_LIMIT):
                        chunk = extra[k: k + _NOP_LIMIT]
                        uid[0] += 1
                        nop = mybir.InstNoOp(
                            name=f"legalize-wait-{uid[0]}",
                            engine=ins.engine,
                            sync_info=mybir.SyncInfo(on_wait=chunk, on_update=[]),
                            bass_nofuse=True,
                        )
                        out.append(nop)
                        n_split += 1
                    ins.sync_info = mybir.SyncInfo(
                        on_wait=keep,
                        on_update=list(si.on_update) if si is not None else [],
                    )
                    changed = True
                out.append(ins)
            if changed:
                bb.instructions = out
    return n_split


def _build(B_LOC=8, S=2048, D=512, U=512):
    SC = S // 512   # score chunks (4)
    NS = S // 128   # s-chunks (16)
    DC = D // 128   # 4
    UC = U // 128   # 4
    nc = bass.Bass()
    enc_d = nc.dram_tensor("enc", [B_LOC, S, D], F32, kind="ExternalInput")
    dec_d = nc.dram_tensor("dec", [B_LOC, D], F32, kind="ExternalInput")
    w1_d = nc.dram_tensor("W1", [D, U], F32, kind="ExternalInput")
    w2_d = nc.dram_tensor("W2", [D, U], F32, kind="ExternalInput")
    v_d = nc.dram_tensor("V", [U], F32, kind="ExternalInput")
    b1_d = nc.dram_tensor("b1", [U], F32, kind="ExternalInput")
    b2_d = nc.dram_tensor("b2", [U], F32, kind="ExternalInput")
    eye_d = nc.dram_tensor("eye", [128, 128], F32, kind="ExternalInput")
    ctx_d = nc.dram_tensor("ctx", [B_LOC, D], F32, kind="ExternalOutput")
    attn_d = nc.dram_tensor("attn", [B_LOC, S], F32, kind="ExternalOutput")

    with tile.TileContext(nc) as tc, contextlib.ExitStack() as cm:
        const = cm.enter_context(tc.tile_pool(name="const", bufs=1))
        psA = cm.enter_context(tc.tile_pool(name="psA", bufs=1, space="PSUM"))

        eyeF = const.tile([128, 128], F32)
        nc.sync.dma_start(eyeF[:], eye_d[:])
        eyeR = const.tile([128, 128], F32R)
        nc.vector.tensor_copy(eyeR[:], eyeF[:])
        onesF = const.tile([128, 1], F32)
        nc.gpsimd.memset(onesF[:], 1.0)
        w1r = []
        vr = const.tile([128, UC], F32R)
        b12 = const.tile([128, UC], F32)
        pdbias = []
        for u in range(UC):
            pdb_t = const.tile([128, B_LOC], F32, tag=f"pd{u}", name=f"pdbias{u}")
            pdbias.append(pdb_t)

        with tc.tile_pool(name="setup", bufs=1) as setup:
            for dc in range(DC):
                st = setup.tile([128, U], F32, tag=f"w1s{dc}")
                nc.sync.dma_start(st[:], w1_d[dc * 128:(dc + 1) * 128, :])
                t = const.tile([128, U], F32R, tag=f"w1r{dc}")
                nc.vector.tensor_copy(t[:], st[:])
                w1r.append(t)
            vstage = setup.tile([128, UC], F32, tag="small")
            nc.sync.dma_start(vstage[:], v_d.rearrange("(c p) -> p c", p=128))
            nc.vector.tensor_copy(vr[:], vstage[:])
            b1t = setup.tile([128, UC], F32, tag="smallb1")
            nc.sync.dma_start(b1t[:], b1_d.rearrange("(c p) -> p c", p=128))
            b2t = setup.tile([128, UC], F32, tag="smallb2")
            nc.sync.dma_start(b2t[:], b2_d.rearrange("(c p) -> p c", p=128))
            nc.vector.tensor_add(b12[:], b1t[:], b2t[:])

            # projdec (plain fp32): pdbias[uc][128(u), B_LOC] = W2.T@dec.T + b12
            decf = setup.tile([B_LOC, D], F32, tag="decf")
            nc.sync.dma_start(decf[:], dec_d[:])
            decTs = []
            for dc in range(DC):
                ps = psA.tile([128, B_LOC], F32, tag="psA")
                nc.tensor.transpose(
                    ps[:], decf[:, dc * 128:(dc + 1) * 128], eyeF[0:B_LOC, 0:B_LOC]
                )
                t = setup.tile([128, B_LOC], F32, tag=f"decT{dc}")
                nc.vector.tensor_copy(t[:], ps[:])
                decTs.append(t)
            for uc in range(UC):
                ps = psA.tile([128, B_LOC], F32, tag="psA")
                for dc in range(DC):
                    st = setup.tile([128, 128], F32, tag=f"w2s{dc}_{uc % 2}")
                    nc.sync.dma_start(
                        st[:], w2_d[dc * 128:(dc + 1) * 128,
                                    uc * 128:(uc + 1) * 128]
                    )
                    nc.tensor.matmul(
                        ps[:], st[:], decTs[dc][:],
                        start=(dc == 0), stop=(dc == DC - 1),
                    )
                nc.vector.tensor_scalar_add(pdbias[uc][:], ps[:], b12[:, uc:uc + 1])

        encn = cm.enter_context(tc.tile_pool(name="encn", bufs=3))
        encnr = cm.enter_context(tc.tile_pool(name="encnr", bufs=2))
        enct = cm.enter_context(tc.tile_pool(name="enct", bufs=2))
        feat = cm.enter_context(tc.tile_pool(name="feat", bufs=2))
        sm1 = cm.enter_context(tc.tile_pool(name="sm1", bufs=1))
        sm2 = cm.enter_context(tc.tile_pool(name="sm2", bufs=2))
        psT = cm.enter_context(tc.tile_pool(name="psT", bufs=2, space="PSUM"))
        psF = cm.enter_context(tc.tile_pool(name="psF", bufs=2, space="PSUM"))
        psS = cm.enter_context(tc.tile_pool(name="psS", bufs=1, space="PSUM"))
        psC = cm.enter_context(tc.tile_pool(name="psC", bufs=1, space="PSUM"))

        for b in range(B_LOC):
            encNr = []
            for sc in range(NS):
                st = encn.tile([128, D], F32, tag="encn")
                nc.sync.dma_start(st[:], enc_d[b, sc * 128:(sc + 1) * 128, :])
                t = encnr.tile([128, D], F32R, tag=f"encnr{sc}")
                nc.vector.tensor_copy(t[:], st[:])
                encNr.append(t)
            encT = [[None] * SC for _ in range(DC)]
            for dc in range(DC):
                for s4 in range(SC):
                    ps = psT.tile([128, 512], F32R, tag="psT")
                    for q in range(4):
                        nc.tensor.transpose(
                            ps[:, q * 128:(q + 1) * 128],
                            encNr[s4 * 4 + q][:, dc * 128:(dc + 1) * 128],
                            eyeR[:],
                        )
                    t = enct.tile([128, 512], F32R, tag=f"enct{dc}_{s4}")
                    nc.vector.tensor_copy(t[:], ps[:])
                    encT[dc][s4] = t
            escore = []
            for s4 in range(SC):
                pss = psS.tile([1, 512], F32, tag="psS")
                for uc in range(UC):
                    psf = psF.tile([128, 512], F32, tag="psF")
                    for dc in range(DC):
                        nc.tensor.matmul(
                            psf[:], w1r[dc][:, uc * 128:(uc + 1) * 128],
                            encT[dc][s4][:],
                            start=(dc == 0), stop=(dc == DC - 1),
                        )
                    ft = feat.tile([128, 512], F32R, tag=f"feat{uc}")
                    nc.scalar.activation(
                        ft[:], psf[:], AF.Tanh, bias=pdbias[uc][:, b:b + 1]
                    )
                    nc.tensor.matmul(
                        pss[:], vr[:, uc:uc + 1], ft[:],
                        start=(uc == 0), stop=(uc == UC - 1),
                    )
                es = sm1.tile([1, 512], F32, tag=f"escore{s4}")
                nc.scalar.activation(es[:], pss[:], AF.Exp)
                escore.append(es)
            psa = psA.tile([128, NS], F32, tag="psA")
            for s4 in range(SC):
                for q in range(4):
                    j = s4 * 4 + q
                    nc.tensor.transpose(
                        psa[:, j:j + 1],
                        escore[s4][:, q * 128:(q + 1) * 128],
                        eyeF[0:1, 0:1],
                    )
            attnTu = sm1.tile([128, NS], F32, tag="attnTu")
            nc.vector.tensor_copy(attnTu[:], psa[:])
            sums = sm1.tile([128, 1], F32, tag="sums")
            nc.vector.reduce_sum(sums[:], attnTu[:], axis=AX.X)
            psr = psC.tile([1, 1], F32, tag="psR")
            nc.tensor.matmul(psr[:], onesF[:], sums[:], start=True, stop=True)
            recip = sm1.tile([1, 1], F32, tag="recip")
            nc.vector.reciprocal(recip[:], psr[:])
            for s4 in range(SC):
                ar = sm2.tile([1, 512], F32, tag="attnR")
                nc.vector.tensor_scalar_mul(ar[:], escore[s4][:], recip[:])
                nc.sync.dma_start(attn_d[b:b + 1, s4 * 512:(s4 + 1) * 512], ar[:])
            attnT = sm1.tile([128, NS], F32R, tag="attnT")
            nc.vector.tensor_copy(attnT[:], attnTu[:])
            psc = psC.tile([1, D], F32, tag="psC")
            for j in range(NS):
                nc.tensor.matmul(
                    psc[:], attnT[:, j:j + 1], encNr[j][:],
                    start=(j == 0), stop=(j == NS - 1),
                )
            ctxrow = sm2.tile([1, D], F32, tag="ctxrow")
            nc.vector.tensor_scalar_mul(ctxrow[:], psc[:], recip[:])
            nc.sync.dma_start(ctx_d[b:b + 1, :], ctxrow[:])

    _legalize_sync_waits(nc)
    return nc


_NC_CACHE = None


def _get_nc():
    global _NC_CACHE
    if _NC_CACHE is None:
        _NC_CACHE = _build(B_LOC, S, D_ENC, UNITS)
    return _NC_CACHE


def kernel(encoder_outputs, decoder_hidden, W1, b1, W2, b2, V, bv):
    enc = np.ascontiguousarray(encoder_outputs, dtype=np.float32)
    dec = np.ascontiguousarray(decoder_hidden, dtype=np.float32)
    nc = _get_nc()
    eye = np.eye(128, dtype=np.float32)
    base = {
        "W1": np.ascontiguousarray(W1, dtype=np.float32),
        "W2": np.ascontiguousarray(W2, dtype=np.float32),
        "V": np.ascontiguousarray(V, dtype=np.float32).reshape(-1),
        "b1": np.ascontiguousarray(b1, dtype=np.float32),
        "b2": np.ascontiguousarray(b2, dtype=np.float32),
        "eye": eye,
    }
    in_maps = []
    for c in range(N_CORES):
        sl = slice(c * B_LOC, (c + 1) * B_LOC)
        in_maps.append(
            {"enc": enc[sl], "dec": dec[sl], **base}
        )
    res = run_bass_kernel_spmd(nc, in_maps, list(range(N_CORES)))
    ctx = np.concatenate([r["ctx"] for r in res.results], axis=0)
    attn = np.concatenate([r["attn"] for r in res.results], axis=0)
    # bv shifts every score equally; softmax is shift-invariant, so attn and
    # context are unchanged — no compute needed for it.
    return ctx, attn
